# revision 13
# baseline (speedup 1.0000x reference)
"""Neighbor aggregation (gnn message passing) Bass kernel for Trainium2.

out[b, i] = sum_{e: src[e]==i} w[e] * H[b, dst[e]]   (per batch b)

8 NeuronCores: core = 2*b + s handles batch b, src-half s (output rows
[s*25000, (s+1)*25000)).

Strategy ("bin-packed one-hot scatter"):
 - The only per-edge data-dependent hardware mechanism is the SWDGE
   dma_gather, whose Q7 descriptor generation costs ~7.9 ns/token on one
   queue and is the hard floor.  The previous kernel paid that floor TWICE
   (gather + dma_scatter_add).  This kernel pays it once — the
   scatter/segment-sum runs on the otherwise-idle Tensor engine as one-hot
   matmuls — and spreads desc-gen over all 4 SWDGE queues (concurrent Q7
   generation contexts, ~3.3 ns/token effective).  8192-token calls need
   single_packet=False (64 desc/engine/packet limit crashes otherwise).
 - Host packs the 25000 output rows of each core into NBINS bins of <=128
   sources, balancing per-bin token counts for BOTH dst-half phases
   (<= TPB*128 tokens per bin per phase).  The resulting tile->bin map is a
   compile-time constant shared by all 8 SPMD cores; all per-core variation
   (gather indices, weights, slot-in-bin ids) is data.
 - Device: per 8192-token chunk: SWDGE dma_gather (HBM H rows -> SBUF
   token-major f32), DVE multiply by w (f32 -> bf16), then per 128-token
   tile: DVE is_equal against a constant iota row builds the one-hot
   [token, slot] matrix; TensorE matmul-accumulates one-hot.T @ msgs into a
   per-bin PSUM tile; after the bin's last tile, DVE adds PSUM into the SBUF
   accumulator [128 slots, NBINS, 64].  Host un-permutes (slot, bin) -> row.
 - Pad tokens gather row 0 with w=0 and slot=-1 (no one-hot match), so they
   are exact no-ops.  Both phases of a source share (slot, bin), so phase
   partials merge in the accumulator with no extra pass.
"""

import os
import sys

sys.path.insert(0, "/opt/trn_rl_repo")

import numpy as np
import ml_dtypes

import concourse.bacc as bacc
import concourse.mybir as mybir
import concourse.tile as tile
from concourse.bass_utils import run_bass_kernel_spmd

B, N, E, HS = 4, 50000, 800000, 64
NHALF = N // 2                  # 25000 output rows per core
CH = 8192                       # tokens per gather chunk
TPB = 7                         # tiles (of 128 tokens) per bin per phase

LAST_RESULT = {}


def build(nc, nbins, nch_per_phase):
    f32 = mybir.dt.float32
    bf16 = mybir.dt.bfloat16
    i16 = mybir.dt.int16
    nch = 2 * nch_per_phase
    tiles_per_phase = nch_per_phase * (CH // 128)
    real_tiles = nbins * TPB    # remaining tiles of the phase are dummies

    h_d = nc.dram_tensor("h", [N, HS], f32, kind="ExternalInput")
    gidx_d = nc.dram_tensor("gidx", [nch, 128, CH // 16], i16, kind="ExternalInput")
    wl_d = nc.dram_tensor("wl", [nch, 128, CH // 128], f32, kind="ExternalInput")
    scol_d = nc.dram_tensor("scol", [nch, 128, CH // 128], bf16, kind="ExternalInput")
    iotab_d = nc.dram_tensor("iotab", [128, 128], bf16, kind="ExternalInput")
    acc_d = nc.dram_tensor("acc", [128, nbins + 1, HS], f32, kind="ExternalOutput")

    with tile.TileContext(nc) as tc:
        with tc.tile_pool(name="res", bufs=1) as res, \
             tc.tile_pool(name="psum", bufs=6, space="PSUM") as pp, \
             tc.tile_pool(name="work", bufs=3) as wp, \
             tc.tile_pool(name="oh", bufs=4) as ohp:
            iotab = res.tile([128, 128], bf16, tag="iotab")
            nc.sync.dma_start(iotab[:], iotab_d[:])
            acc = res.tile([128, nbins + 1, HS], f32, tag="acc")
            nc.vector.memset(acc[:], 0.0)

            ps = None
            for c in range(nch):
                phase = c // nch_per_phase
                h_ap = h_d[:][phase * NHALF:(phase + 1) * NHALF, :]
                gi = wp.tile([128, CH // 16], i16, tag="gi")
                wl = wp.tile([128, CH // 128], f32, tag="wl")
                sc = wp.tile([128, CH // 128], bf16, tag="sc")
                nc.sync.dma_start(gi[:], gidx_d[c])
                nc.sync.dma_start(wl[:], wl_d[c])
                nc.sync.dma_start(sc[:], scol_d[c])

                msgs = wp.tile([128, CH // 128, HS], f32, tag="msgs")
                nc.gpsimd.dma_gather(
                    out_ap=msgs[:],
                    in_ap=h_ap,
                    idxs_ap=gi[:],
                    num_idxs=CH,
                    num_idxs_reg=CH,
                    elem_size=HS,
                    single_packet=False,
                    queue_num=c % 4,
                )
                msgsb = wp.tile([128, CH // 128, HS], bf16, tag="msgsb")
                nc.vector.tensor_tensor(
                    out=msgsb[:],
                    in0=msgs[:],
                    in1=wl[:].unsqueeze(2).broadcast_to([128, CH // 128, HS]),
                    op=mybir.AluOpType.mult,
                )

                ntile = CH // 128
                for j0 in range(0, ntile, 8):
                    nb = min(8, ntile - j0)
                    oh = ohp.tile([128, 8, 128], bf16, tag="oh")
                    nc.vector.tensor_tensor(
                        out=oh[:, :nb],
                        in0=sc[:, j0:j0 + nb].unsqueeze(2).broadcast_to([128, nb, 128]),
                        in1=iotab[:].unsqueeze(1).broadcast_to([128, nb, 128]),
                        op=mybir.AluOpType.is_equal,
                    )
                    for j in range(j0, j0 + nb):
                        tau = (c % nch_per_phase) * ntile + j   # tile idx in phase
                        if tau < real_tiles:
                            bin_, pos = tau // TPB, tau % TPB
                            last = pos == TPB - 1
                        else:
                            bin_, pos = nbins, tau - real_tiles  # dummy bin
                            last = tau == tiles_per_phase - 1
                        if pos == 0:
                            ps = pp.tile([128, HS], f32, tag="ps")
                        nc.tensor.matmul(
                            ps[:], oh[:, j - j0], msgsb[:, j],
                            start=(pos == 0), stop=last,
                        )
                        if last:
                            nc.vector.tensor_tensor(
                                out=acc[:, bin_], in0=acc[:, bin_], in1=ps[:],
                                op=mybir.AluOpType.add,
                            )
                            # bins complete in ascending order during phase B:
                            # stream finished 32-bin blocks out early
                            if phase == 1 and bin_ < nbins and bin_ % 32 == 31:
                                nc.sync.dma_start(
                                    acc_d[:][:, bin_ - 31:bin_ + 1],
                                    acc[:, bin_ - 31:bin_ + 1],
                                )

            rem = nbins - (nbins // 32) * 32
            nc.sync.dma_start(
                acc_d[:][:, nbins - rem:], acc[:, nbins - rem:]
            )
    return nc


_COMPILED = {}


def _get_compiled(nbins, nch_per_phase):
    key = (nbins, nch_per_phase)
    if key not in _COMPILED:
        nc = bacc.Bacc(
            "TRN2", target_bir_lowering=False, debug=False, num_swdge_queues=4
        )
        build(nc, nbins, nch_per_phase)
        nc.compile()
        _COMPILED[key] = nc
    return _COMPILED[key]


def _pack_bins(dA, dB, nbins, cap):
    """Assign each source to a bin s.t. per-bin source count <=128 and
    per-bin token sums <= cap in BOTH phases.  Returns (bin, slot) per
    source, or None if infeasible."""
    nsrc = dA.shape[0]
    order = np.argsort(-(dA + dB), kind="stable")
    loadA = np.zeros(nbins, np.int64)
    loadB = np.zeros(nbins, np.int64)
    cnt = np.zeros(nbins, np.int64)
    bin_of = np.empty(nsrc, np.int64)
    slot_of = np.empty(nsrc, np.int64)
    for s in order:
        headA = cap - loadA - dA[s]
        headB = cap - loadB - dB[s]
        score = np.minimum(headA, headB)
        score[cnt >= 128] = -1
        b = int(np.argmax(score))
        if score[b] < 0:
            return None
        bin_of[s] = b
        slot_of[s] = cnt[b]
        loadA[b] += dA[s]
        loadB[b] += dB[s]
        cnt[b] += 1
    return bin_of, slot_of


def _wrap16(idx, n):
    a = idx.reshape(n // 16, 16).T.astype(np.int16)   # [16, n//16]
    return np.ascontiguousarray(np.tile(a, (8, 1)))   # [128, n//16]


def _core_edges(src, dst, w, s):
    sel = (src >= NHALF) == bool(s)
    srcs = (src[sel] - s * NHALF).astype(np.int64)
    dsts = dst[sel].astype(np.int64)
    ws = w[sel].astype(np.float32)
    phase = (dsts >= NHALF).astype(np.int64)
    dloc = dsts - phase * NHALF
    return srcs, dloc, ws, phase


def _prep_core(srcs, dloc, ws, phase, bin_of, slot_of, nbins, nch_per_phase):
    """Build gidx/wl/scol chunk arrays for one core (batch half s)."""
    cap = TPB * 128
    ntok = nch_per_phase * CH
    g_all = np.zeros((2, ntok), np.int64)
    w_all = np.zeros((2, ntok), np.float32)
    s_all = np.full((2, ntok), -1.0, np.float32)

    for ph in range(2):
        m = phase == ph
        sp, dp, wp_ = srcs[m], dloc[m], ws[m]
        # order edges by bin: position = bin base + running offset within bin
        b = bin_of[sp]
        order = np.argsort(b, kind="stable")
        sp, dp, wp_, b = sp[order], dp[order], wp_[order], b[order]
        cnts = np.bincount(b, minlength=nbins)
        starts = np.concatenate([[0], np.cumsum(cnts[:-1])])
        offs = np.arange(sp.shape[0]) - np.repeat(starts, cnts)
        pos = b * cap + offs
        assert (offs < cap).all()
        g_all[ph, pos] = dp
        w_all[ph, pos] = wp_
        s_all[ph, pos] = slot_of[sp]

    gidx = np.stack([
        _wrap16(g_all[ph, c * CH:(c + 1) * CH], CH)
        for ph in range(2) for c in range(nch_per_phase)
    ])
    # token t of chunk -> [t % 128, t // 128]
    wl = np.ascontiguousarray(
        w_all.reshape(2 * nch_per_phase, CH // 128, 128).transpose(0, 2, 1))
    scol = np.ascontiguousarray(
        s_all.reshape(2 * nch_per_phase, CH // 128, 128).transpose(0, 2, 1)
    ).astype(ml_dtypes.bfloat16)
    return {"gidx": gidx, "wl": wl, "scol": scol}


def kernel(**inputs):
    H = np.ascontiguousarray(np.asarray(inputs["H"], np.float32))
    w = np.asarray(inputs["edge_w"], np.float32)
    src = np.asarray(inputs["edge_src"], np.int64)
    dst = np.asarray(inputs["edge_dst"], np.int64)

    cap = TPB * 128
    edges = []
    worst = 1
    for core in range(8):
        b, s = core // 2, core % 2
        srcs, dloc, ws, phase = _core_edges(src[b], dst[b], w[b], s)
        edges.append((srcs, dloc, ws, phase))
        worst = max(worst, int((phase == 0).sum()), int((phase == 1).sum()))

    # pack all cores; grow nbins until feasible everywhere
    nbins = max(-(-NHALF // 128), -(-int(worst * 1.01) // cap))
    nbins = -(-nbins // 4) * 4
    while True:
        metas = []
        for core in range(8):
            srcs, dloc, ws, phase = edges[core]
            dA = np.bincount(srcs[phase == 0], minlength=NHALF)
            dB = np.bincount(srcs[phase == 1], minlength=NHALF)
            res = _pack_bins(dA, dB, nbins, cap)
            if res is None:
                break
            metas.append(res)
        if len(metas) == 8:
            break
        nbins += 4
    nch_per_phase = -(-(nbins * cap) // CH)

    iotab = np.tile(np.arange(128), (128, 1)).astype(ml_dtypes.bfloat16)

    in_maps = []
    for core in range(8):
        b = core // 2
        srcs, dloc, ws, phase = edges[core]
        bin_of, slot_of = metas[core]
        m = _prep_core(srcs, dloc, ws, phase, bin_of, slot_of, nbins, nch_per_phase)
        m["h"] = H[b]
        m["iotab"] = iotab
        in_maps.append(m)

    nc = _get_compiled(nbins, nch_per_phase)
    trace = bool(int(os.environ.get("GNN_TRACE", "0")))
    res = run_bass_kernel_spmd(nc, in_maps, list(range(8)), trace=trace)
    LAST_RESULT["exec_time_ns"] = res.exec_time_ns
    LAST_RESULT["res"] = res

    out = np.empty((B, N, HS), np.float32)
    rows = np.arange(NHALF)
    for core in range(8):
        b, s = core // 2, core % 2
        bin_of, slot_of = metas[core]
        dump = res.results[core]["acc"]          # [128, nbins+1, 64]
        out[b, s * NHALF:(s + 1) * NHALF] = dump[slot_of[rows], bin_of[rows]]
    return out


# revision 14
# speedup vs baseline: 1.1369x; 1.1369x over previous
"""Neighbor aggregation (gnn message passing) Bass kernel for Trainium2.

out[b, i] = sum_{e: src[e]==i} w[e] * H[b, dst[e]]   (per batch b)

8 NeuronCores: core = 2*b + s handles batch b, src-half s (output rows
[s*25000, (s+1)*25000)).

Strategy ("bin-packed one-hot scatter"):
 - The only per-edge data-dependent hardware mechanism is the SWDGE
   dma_gather, whose Q7 descriptor generation costs ~7.9 ns/token on one
   queue and is the hard floor.  The previous kernel paid that floor TWICE
   (gather + dma_scatter_add).  This kernel pays it once — the
   scatter/segment-sum runs on the otherwise-idle Tensor engine as one-hot
   matmuls — and spreads desc-gen over all 4 SWDGE queues (concurrent Q7
   generation contexts, ~3.3 ns/token effective).  8192-token calls need
   single_packet=False (64 desc/engine/packet limit crashes otherwise).
 - Host packs the 25000 output rows of each core into NBINS bins of <=128
   sources, balancing per-bin token counts for BOTH dst-half phases
   (<= TPB*128 tokens per bin per phase).  The resulting tile->bin map is a
   compile-time constant shared by all 8 SPMD cores; all per-core variation
   (gather indices, weights, slot-in-bin ids) is data.
 - Device: per 8192-token chunk: SWDGE dma_gather (HBM H rows -> SBUF
   token-major f32), DVE multiply by w (f32 -> bf16), then per 128-token
   tile: DVE is_equal against a constant iota row builds the one-hot
   [token, slot] matrix; TensorE matmul-accumulates one-hot.T @ msgs into a
   per-bin PSUM tile; after the bin's last tile, DVE adds PSUM into the SBUF
   accumulator [128 slots, NBINS, 64].  Host un-permutes (slot, bin) -> row.
 - Pad tokens gather row 0 with w=0 and slot=-1 (no one-hot match), so they
   are exact no-ops.  Both phases of a source share (slot, bin), so phase
   partials merge in the accumulator with no extra pass.
"""

import os
import sys

sys.path.insert(0, "/opt/trn_rl_repo")

import numpy as np
import ml_dtypes

import concourse.bacc as bacc
import concourse.mybir as mybir
import concourse.tile as tile
from concourse.bass_utils import run_bass_kernel_spmd

B, N, E, HS = 4, 50000, 800000, 64
NHALF = N // 2                  # 25000 output rows per core
CH = 8192                       # tokens per gather chunk
TPB = 7                         # tiles (of 128 tokens) per bin per phase

LAST_RESULT = {}


def build(nc, nbins, nch_per_phase):
    f32 = mybir.dt.float32
    bf16 = mybir.dt.bfloat16
    i16 = mybir.dt.int16
    nch = 2 * nch_per_phase
    tiles_per_phase = nch_per_phase * (CH // 128)
    real_tiles = nbins * TPB    # remaining tiles of the phase are dummies

    h_d = nc.dram_tensor("h", [N, HS], f32, kind="ExternalInput")
    gidx_d = nc.dram_tensor("gidx", [nch, 128, CH // 16], i16, kind="ExternalInput")
    wl_d = nc.dram_tensor("wl", [nch, 128, CH // 128], f32, kind="ExternalInput")
    scol_d = nc.dram_tensor("scol", [nch, 128, CH // 128], bf16, kind="ExternalInput")
    iotab_d = nc.dram_tensor("iotab", [128, 128], bf16, kind="ExternalInput")
    acc_d = nc.dram_tensor("acc", [128, nbins + 1, HS], f32, kind="ExternalOutput")

    with tile.TileContext(nc) as tc:
        with tc.tile_pool(name="res", bufs=1) as res, \
             tc.tile_pool(name="psum", bufs=8, space="PSUM") as pp, \
             tc.tile_pool(name="work", bufs=4) as wp, \
             tc.tile_pool(name="oh", bufs=4) as ohp:
            iotab = res.tile([128, 128], bf16, tag="iotab")
            nc.sync.dma_start(iotab[:], iotab_d[:])
            acc = res.tile([128, nbins + 1, HS], f32, tag="acc")
            nc.vector.memset(acc[:], 0.0)

            ps = None
            for c in range(nch):
                phase = c // nch_per_phase
                h_ap = h_d[:][phase * NHALF:(phase + 1) * NHALF, :]
                gi = wp.tile([128, CH // 16], i16, tag="gi")
                wl = wp.tile([128, CH // 128], f32, tag="wl")
                sc = wp.tile([128, CH // 128], bf16, tag="sc")
                nc.sync.dma_start(gi[:], gidx_d[c])
                nc.sync.dma_start(wl[:], wl_d[c])
                nc.sync.dma_start(sc[:], scol_d[c])

                msgs = wp.tile([128, CH // 128, HS], f32, tag="msgs")
                nc.gpsimd.dma_gather(
                    out_ap=msgs[:],
                    in_ap=h_ap,
                    idxs_ap=gi[:],
                    num_idxs=CH,
                    num_idxs_reg=CH,
                    elem_size=HS,
                    single_packet=False,
                    queue_num=c % 4,
                )
                msgsb = wp.tile([128, CH // 128, HS], bf16, tag="msgsb")
                nc.vector.tensor_tensor(
                    out=msgsb[:],
                    in0=msgs[:],
                    in1=wl[:].unsqueeze(2).broadcast_to([128, CH // 128, HS]),
                    op=mybir.AluOpType.mult,
                )

                ntile = CH // 128
                for j0 in range(0, ntile, 8):
                    nb = min(8, ntile - j0)
                    oh = ohp.tile([128, 8, 128], bf16, tag="oh")
                    nc.vector.tensor_tensor(
                        out=oh[:, :nb],
                        in0=sc[:, j0:j0 + nb].unsqueeze(2).broadcast_to([128, nb, 128]),
                        in1=iotab[:].unsqueeze(1).broadcast_to([128, nb, 128]),
                        op=mybir.AluOpType.is_equal,
                    )
                    for j in range(j0, j0 + nb):
                        tau = (c % nch_per_phase) * ntile + j   # tile idx in phase
                        if tau < real_tiles:
                            bin_, pos = tau // TPB, tau % TPB
                            last = pos == TPB - 1
                        else:
                            bin_, pos = nbins, tau - real_tiles  # dummy bin
                            last = tau == tiles_per_phase - 1
                        if pos == 0:
                            ps = pp.tile([128, HS], f32, tag="ps")
                        nc.tensor.matmul(
                            ps[:], oh[:, j - j0], msgsb[:, j],
                            start=(pos == 0), stop=last,
                        )
                        if last:
                            nc.vector.tensor_tensor(
                                out=acc[:, bin_], in0=acc[:, bin_], in1=ps[:],
                                op=mybir.AluOpType.add,
                            )
                            # bins complete in ascending order during phase B:
                            # stream finished 32-bin blocks out early
                            if phase == 1 and bin_ < nbins and bin_ % 32 == 31:
                                nc.sync.dma_start(
                                    acc_d[:][:, bin_ - 31:bin_ + 1],
                                    acc[:, bin_ - 31:bin_ + 1],
                                )

            rem = nbins - (nbins // 32) * 32
            nc.sync.dma_start(
                acc_d[:][:, nbins - rem:], acc[:, nbins - rem:]
            )
    return nc


_COMPILED = {}


def _get_compiled(nbins, nch_per_phase):
    key = (nbins, nch_per_phase)
    if key not in _COMPILED:
        nc = bacc.Bacc(
            "TRN2", target_bir_lowering=False, debug=False, num_swdge_queues=4
        )
        build(nc, nbins, nch_per_phase)
        nc.compile()
        _COMPILED[key] = nc
    return _COMPILED[key]


def _pack_bins(dA, dB, nbins, cap):
    """Assign each source to a bin s.t. per-bin source count <=128 and
    per-bin token sums <= cap in BOTH phases.  Returns (bin, slot) per
    source, or None if infeasible."""
    nsrc = dA.shape[0]
    order = np.argsort(-(dA + dB), kind="stable")
    loadA = np.zeros(nbins, np.int64)
    loadB = np.zeros(nbins, np.int64)
    cnt = np.zeros(nbins, np.int64)
    bin_of = np.empty(nsrc, np.int64)
    slot_of = np.empty(nsrc, np.int64)
    for s in order:
        headA = cap - loadA - dA[s]
        headB = cap - loadB - dB[s]
        score = np.minimum(headA, headB)
        score[cnt >= 128] = -1
        b = int(np.argmax(score))
        if score[b] < 0:
            return None
        bin_of[s] = b
        slot_of[s] = cnt[b]
        loadA[b] += dA[s]
        loadB[b] += dB[s]
        cnt[b] += 1
    return bin_of, slot_of


def _wrap16(idx, n):
    a = idx.reshape(n // 16, 16).T.astype(np.int16)   # [16, n//16]
    return np.ascontiguousarray(np.tile(a, (8, 1)))   # [128, n//16]


def _core_edges(src, dst, w, s):
    sel = (src >= NHALF) == bool(s)
    srcs = (src[sel] - s * NHALF).astype(np.int64)
    dsts = dst[sel].astype(np.int64)
    ws = w[sel].astype(np.float32)
    phase = (dsts >= NHALF).astype(np.int64)
    dloc = dsts - phase * NHALF
    return srcs, dloc, ws, phase


def _prep_core(srcs, dloc, ws, phase, bin_of, slot_of, nbins, nch_per_phase):
    """Build gidx/wl/scol chunk arrays for one core (batch half s)."""
    cap = TPB * 128
    ntok = nch_per_phase * CH
    g_all = np.zeros((2, ntok), np.int64)
    w_all = np.zeros((2, ntok), np.float32)
    s_all = np.full((2, ntok), -1.0, np.float32)

    for ph in range(2):
        m = phase == ph
        sp, dp, wp_ = srcs[m], dloc[m], ws[m]
        # order edges by bin: position = bin base + running offset within bin
        b = bin_of[sp]
        order = np.argsort(b, kind="stable")
        sp, dp, wp_, b = sp[order], dp[order], wp_[order], b[order]
        cnts = np.bincount(b, minlength=nbins)
        starts = np.concatenate([[0], np.cumsum(cnts[:-1])])
        offs = np.arange(sp.shape[0]) - np.repeat(starts, cnts)
        pos = b * cap + offs
        assert (offs < cap).all()
        g_all[ph, pos] = dp
        w_all[ph, pos] = wp_
        s_all[ph, pos] = slot_of[sp]

    gidx = np.stack([
        _wrap16(g_all[ph, c * CH:(c + 1) * CH], CH)
        for ph in range(2) for c in range(nch_per_phase)
    ])
    # token t of chunk -> [t % 128, t // 128]
    wl = np.ascontiguousarray(
        w_all.reshape(2 * nch_per_phase, CH // 128, 128).transpose(0, 2, 1))
    scol = np.ascontiguousarray(
        s_all.reshape(2 * nch_per_phase, CH // 128, 128).transpose(0, 2, 1)
    ).astype(ml_dtypes.bfloat16)
    return {"gidx": gidx, "wl": wl, "scol": scol}


def kernel(**inputs):
    H = np.ascontiguousarray(np.asarray(inputs["H"], np.float32))
    w = np.asarray(inputs["edge_w"], np.float32)
    src = np.asarray(inputs["edge_src"], np.int64)
    dst = np.asarray(inputs["edge_dst"], np.int64)

    cap = TPB * 128
    edges = []
    worst = 1
    for core in range(8):
        b, s = core // 2, core % 2
        srcs, dloc, ws, phase = _core_edges(src[b], dst[b], w[b], s)
        edges.append((srcs, dloc, ws, phase))
        worst = max(worst, int((phase == 0).sum()), int((phase == 1).sum()))

    # pack all cores; grow nbins until feasible everywhere
    nbins = max(-(-NHALF // 128), -(-int(worst * 1.01) // cap))
    nbins = -(-nbins // 4) * 4
    while True:
        metas = []
        for core in range(8):
            srcs, dloc, ws, phase = edges[core]
            dA = np.bincount(srcs[phase == 0], minlength=NHALF)
            dB = np.bincount(srcs[phase == 1], minlength=NHALF)
            res = _pack_bins(dA, dB, nbins, cap)
            if res is None:
                break
            metas.append(res)
        if len(metas) == 8:
            break
        nbins += 4
    nch_per_phase = -(-(nbins * cap) // CH)

    iotab = np.tile(np.arange(128), (128, 1)).astype(ml_dtypes.bfloat16)

    in_maps = []
    for core in range(8):
        b = core // 2
        srcs, dloc, ws, phase = edges[core]
        bin_of, slot_of = metas[core]
        m = _prep_core(srcs, dloc, ws, phase, bin_of, slot_of, nbins, nch_per_phase)
        m["h"] = H[b]
        m["iotab"] = iotab
        in_maps.append(m)

    nc = _get_compiled(nbins, nch_per_phase)
    trace = bool(int(os.environ.get("GNN_TRACE", "0")))
    res = run_bass_kernel_spmd(nc, in_maps, list(range(8)), trace=trace)
    LAST_RESULT["exec_time_ns"] = res.exec_time_ns
    LAST_RESULT["res"] = res

    out = np.empty((B, N, HS), np.float32)
    rows = np.arange(NHALF)
    for core in range(8):
        b, s = core // 2, core % 2
        bin_of, slot_of = metas[core]
        dump = res.results[core]["acc"]          # [128, nbins+1, 64]
        out[b, s * NHALF:(s + 1) * NHALF] = dump[slot_of[rows], bin_of[rows]]
    return out


# revision 18
# speedup vs baseline: 1.1480x; 1.0098x over previous
"""Neighbor aggregation (gnn message passing) Bass kernel for Trainium2.

out[b, i] = sum_{e: src[e]==i} w[e] * H[b, dst[e]]   (per batch b)

8 NeuronCores: core = 2*b + s handles batch b, src-half s (output rows
[s*25000, (s+1)*25000)).

Strategy ("bin-packed one-hot scatter"):
 - The only per-edge data-dependent hardware mechanism is the SWDGE
   dma_gather, whose Q7 descriptor generation costs ~7.9 ns/token on one
   queue and is the hard floor.  The previous kernel paid that floor TWICE
   (gather + dma_scatter_add).  This kernel pays it once — the
   scatter/segment-sum runs on the otherwise-idle Tensor engine as one-hot
   matmuls — and spreads desc-gen over all 4 SWDGE queues (concurrent Q7
   generation contexts, ~3.3 ns/token effective).  8192-token calls need
   single_packet=False (64 desc/engine/packet limit crashes otherwise).
 - Host packs the 25000 output rows of each core into NBINS bins of <=128
   sources, balancing per-bin token counts for BOTH dst-half phases
   (<= TPB*128 tokens per bin per phase).  The resulting tile->bin map is a
   compile-time constant shared by all 8 SPMD cores; all per-core variation
   (gather indices, weights, slot-in-bin ids) is data.
 - Device: per 8192-token chunk: SWDGE dma_gather (HBM H rows -> SBUF
   token-major f32), DVE multiply by w (f32 -> bf16), then per 128-token
   tile: DVE is_equal against a constant iota row builds the one-hot
   [token, slot] matrix; TensorE matmul-accumulates one-hot.T @ msgs into a
   per-bin PSUM tile; after the bin's last tile, DVE adds PSUM into the SBUF
   accumulator [128 slots, NBINS, 64].  Host un-permutes (slot, bin) -> row.
 - Pad tokens gather row 0 with w=0 and slot=-1 (no one-hot match), so they
   are exact no-ops.  Both phases of a source share (slot, bin), so phase
   partials merge in the accumulator with no extra pass.
"""

import os
import sys

sys.path.insert(0, "/opt/trn_rl_repo")

import numpy as np
import ml_dtypes

import concourse.bacc as bacc
import concourse.mybir as mybir
import concourse.tile as tile
from concourse.bass_utils import run_bass_kernel_spmd

B, N, E, HS = 4, 50000, 800000, 64
NHALF = N // 2                  # 25000 output rows per core
CH = 8192                       # tokens per gather chunk
TPB = 7                         # tiles (of 128 tokens) per bin per phase

LAST_RESULT = {}


def _chunk_sizes(nch_per_phase):
    """Per-phase chunk sizes: 4 small chunks first (fast pipeline fill across
    the 4 SWDGE queues) resp. last (fast drain); same total tokens."""
    sizes_a = [CH // 4] * 4 + [CH] * (nch_per_phase - 1)
    sizes_b = [CH] * (nch_per_phase - 1) + [CH // 4] * 4
    return sizes_a, sizes_b


def build(nc, nbins, nch_per_phase):
    f32 = mybir.dt.float32
    bf16 = mybir.dt.bfloat16
    i16 = mybir.dt.int16
    sizes_a, sizes_b = _chunk_sizes(nch_per_phase)
    sizes = sizes_a + sizes_b
    nch = len(sizes)
    tiles_per_phase = nch_per_phase * (CH // 128)
    real_tiles = nbins * TPB    # remaining tiles of the phase are dummies

    h_d = nc.dram_tensor("h", [N, HS], f32, kind="ExternalInput")
    gidx_d = nc.dram_tensor("gidx", [nch, 128, CH // 16], i16, kind="ExternalInput")
    wl_d = nc.dram_tensor("wl", [nch, 128, CH // 128], f32, kind="ExternalInput")
    scol_d = nc.dram_tensor("scol", [nch, 128, CH // 128], bf16, kind="ExternalInput")
    iotab_d = nc.dram_tensor("iotab", [128, 128], bf16, kind="ExternalInput")
    acc_d = nc.dram_tensor("acc", [128, nbins + 1, HS], f32, kind="ExternalOutput")

    with tile.TileContext(nc) as tc:
        with tc.tile_pool(name="res", bufs=1) as res, \
             tc.tile_pool(name="psum", bufs=8, space="PSUM") as pp, \
             tc.tile_pool(name="work", bufs=4) as wp, \
             tc.tile_pool(name="oh", bufs=4) as ohp:
            iotab = res.tile([128, 128], bf16, tag="iotab")
            nc.sync.dma_start(iotab[:], iotab_d[:])
            acc = res.tile([128, nbins + 1, HS], f32, tag="acc")
            nc.vector.memset(acc[:], 0.0)

            ps = None
            off = 0                     # token offset within the phase
            for c, size in enumerate(sizes):
                phase = 0 if c < len(sizes_a) else 1
                if c == len(sizes_a):
                    off = 0
                h_ap = h_d[:][phase * NHALF:(phase + 1) * NHALF, :]
                gi = wp.tile([128, size // 16], i16, tag="gi")
                wl = wp.tile([128, size // 128], f32, tag="wl")
                sc = wp.tile([128, size // 128], bf16, tag="sc")
                nc.sync.dma_start(gi[:], gidx_d[c][:, :size // 16])
                nc.sync.dma_start(wl[:], wl_d[c][:, :size // 128])
                nc.sync.dma_start(sc[:], scol_d[c][:, :size // 128])

                msgs = wp.tile([128, size // 128, HS], f32, tag="msgs")
                nc.gpsimd.dma_gather(
                    out_ap=msgs[:],
                    in_ap=h_ap,
                    idxs_ap=gi[:],
                    num_idxs=size,
                    num_idxs_reg=size,
                    elem_size=HS,
                    single_packet=False,
                    queue_num=c % 4,
                )
                msgsb = wp.tile([128, size // 128, HS], bf16, tag="msgsb")
                nc.vector.tensor_tensor(
                    out=msgsb[:],
                    in0=msgs[:],
                    in1=wl[:].unsqueeze(2).broadcast_to([128, size // 128, HS]),
                    op=mybir.AluOpType.mult,
                )

                ntile = size // 128
                tau0 = off // 128
                for j0 in range(0, ntile, 8):
                    nb = min(8, ntile - j0)
                    oh = ohp.tile([128, 8, 128], bf16, tag="oh")
                    nc.vector.tensor_tensor(
                        out=oh[:, :nb],
                        in0=sc[:, j0:j0 + nb].unsqueeze(2).broadcast_to([128, nb, 128]),
                        in1=iotab[:].unsqueeze(1).broadcast_to([128, nb, 128]),
                        op=mybir.AluOpType.is_equal,
                    )
                    for j in range(j0, j0 + nb):
                        tau = tau0 + j                          # tile idx in phase
                        if tau < real_tiles:
                            bin_, pos = tau // TPB, tau % TPB
                            last = pos == TPB - 1
                        else:
                            bin_, pos = nbins, tau - real_tiles  # dummy bin
                            last = tau == tiles_per_phase - 1
                        if pos == 0:
                            ps = pp.tile([128, HS], f32, tag="ps")
                        nc.tensor.matmul(
                            ps[:], oh[:, j - j0], msgsb[:, j],
                            start=(pos == 0), stop=last,
                        )
                        if last:
                            nc.vector.tensor_tensor(
                                out=acc[:, bin_], in0=acc[:, bin_], in1=ps[:],
                                op=mybir.AluOpType.add,
                            )
                            # bins complete in ascending order during phase B:
                            # stream finished 32-bin blocks out early
                            if phase == 1 and bin_ < nbins and bin_ % 32 == 31:
                                nc.sync.dma_start(
                                    acc_d[:][:, bin_ - 31:bin_ + 1],
                                    acc[:, bin_ - 31:bin_ + 1],
                                )
                off += size

            rem = nbins - (nbins // 32) * 32
            nc.sync.dma_start(
                acc_d[:][:, nbins - rem:], acc[:, nbins - rem:]
            )
    return nc


_COMPILED = {}


def _get_compiled(nbins, nch_per_phase):
    key = (nbins, nch_per_phase)
    if key not in _COMPILED:
        nc = bacc.Bacc(
            "TRN2", target_bir_lowering=False, debug=False, num_swdge_queues=4
        )
        build(nc, nbins, nch_per_phase)
        nc.compile()
        _COMPILED[key] = nc
    return _COMPILED[key]


def _pack_bins(dA, dB, nbins, cap):
    """Assign each source to a bin s.t. per-bin source count <=128 and
    per-bin token sums <= cap in BOTH phases.  Returns (bin, slot) per
    source, or None if infeasible."""
    nsrc = dA.shape[0]
    order = np.argsort(-(dA + dB), kind="stable")
    loadA = np.zeros(nbins, np.int64)
    loadB = np.zeros(nbins, np.int64)
    cnt = np.zeros(nbins, np.int64)
    bin_of = np.empty(nsrc, np.int64)
    slot_of = np.empty(nsrc, np.int64)
    for s in order:
        headA = cap - loadA - dA[s]
        headB = cap - loadB - dB[s]
        score = np.minimum(headA, headB)
        score[cnt >= 128] = -1
        b = int(np.argmax(score))
        if score[b] < 0:
            return None
        bin_of[s] = b
        slot_of[s] = cnt[b]
        loadA[b] += dA[s]
        loadB[b] += dB[s]
        cnt[b] += 1
    return bin_of, slot_of


def _wrap16(idx, n):
    a = idx.reshape(n // 16, 16).T.astype(np.int16)   # [16, n//16]
    return np.ascontiguousarray(np.tile(a, (8, 1)))   # [128, n//16]


def _core_edges(src, dst, w, s):
    sel = (src >= NHALF) == bool(s)
    srcs = (src[sel] - s * NHALF).astype(np.int64)
    dsts = dst[sel].astype(np.int64)
    ws = w[sel].astype(np.float32)
    phase = (dsts >= NHALF).astype(np.int64)
    dloc = dsts - phase * NHALF
    return srcs, dloc, ws, phase


def _prep_core(srcs, dloc, ws, phase, bin_of, slot_of, nbins, nch_per_phase):
    """Build gidx/wl/scol chunk arrays for one core (batch half s)."""
    cap = TPB * 128
    ntok = nch_per_phase * CH
    g_all = np.zeros((2, ntok), np.int64)
    w_all = np.zeros((2, ntok), np.float32)
    s_all = np.full((2, ntok), -1.0, np.float32)

    for ph in range(2):
        m = phase == ph
        sp, dp, wp_ = srcs[m], dloc[m], ws[m]
        # order edges by bin: position = bin base + running offset within bin
        b = bin_of[sp]
        order = np.argsort(b, kind="stable")
        sp, dp, wp_, b = sp[order], dp[order], wp_[order], b[order]
        cnts = np.bincount(b, minlength=nbins)
        starts = np.concatenate([[0], np.cumsum(cnts[:-1])])
        offs = np.arange(sp.shape[0]) - np.repeat(starts, cnts)
        pos = b * cap + offs
        assert (offs < cap).all()
        g_all[ph, pos] = dp
        w_all[ph, pos] = wp_
        s_all[ph, pos] = slot_of[sp]

    sizes_a, sizes_b = _chunk_sizes(nch_per_phase)
    nch = len(sizes_a) + len(sizes_b)
    gidx = np.zeros((nch, 128, CH // 16), np.int16)
    wl = np.zeros((nch, 128, CH // 128), np.float32)
    s_out = np.full((nch, 128, CH // 128), -1.0, np.float32)
    c = 0
    for ph, sizes in ((0, sizes_a), (1, sizes_b)):
        off = 0
        for size in sizes:
            tok = slice(off, off + size)
            gidx[c, :, :size // 16] = _wrap16(g_all[ph, tok], size)
            # token t of chunk -> [t % 128, t // 128]
            wl[c, :, :size // 128] = w_all[ph, tok].reshape(size // 128, 128).T
            s_out[c, :, :size // 128] = s_all[ph, tok].reshape(size // 128, 128).T
            off += size
            c += 1
    scol = s_out.astype(ml_dtypes.bfloat16)
    return {"gidx": gidx, "wl": wl, "scol": scol}


def kernel(**inputs):
    H = np.ascontiguousarray(np.asarray(inputs["H"], np.float32))
    w = np.asarray(inputs["edge_w"], np.float32)
    src = np.asarray(inputs["edge_src"], np.int64)
    dst = np.asarray(inputs["edge_dst"], np.int64)

    cap = TPB * 128
    edges = []
    worst = 1
    for core in range(8):
        b, s = core // 2, core % 2
        srcs, dloc, ws, phase = _core_edges(src[b], dst[b], w[b], s)
        edges.append((srcs, dloc, ws, phase))
        worst = max(worst, int((phase == 0).sum()), int((phase == 1).sum()))

    # pack all cores; grow nbins until feasible everywhere
    nbins = max(-(-NHALF // 128), -(-int(worst * 1.01) // cap))
    nbins = -(-nbins // 4) * 4
    while True:
        metas = []
        for core in range(8):
            srcs, dloc, ws, phase = edges[core]
            dA = np.bincount(srcs[phase == 0], minlength=NHALF)
            dB = np.bincount(srcs[phase == 1], minlength=NHALF)
            res = _pack_bins(dA, dB, nbins, cap)
            if res is None:
                break
            metas.append(res)
        if len(metas) == 8:
            break
        nbins += 4
    nch_per_phase = -(-(nbins * cap) // CH)

    iotab = np.tile(np.arange(128), (128, 1)).astype(ml_dtypes.bfloat16)

    in_maps = []
    for core in range(8):
        b = core // 2
        srcs, dloc, ws, phase = edges[core]
        bin_of, slot_of = metas[core]
        m = _prep_core(srcs, dloc, ws, phase, bin_of, slot_of, nbins, nch_per_phase)
        m["h"] = H[b]
        m["iotab"] = iotab
        in_maps.append(m)

    nc = _get_compiled(nbins, nch_per_phase)
    trace = bool(int(os.environ.get("GNN_TRACE", "0")))
    res = run_bass_kernel_spmd(nc, in_maps, list(range(8)), trace=trace)
    LAST_RESULT["exec_time_ns"] = res.exec_time_ns
    LAST_RESULT["res"] = res

    out = np.empty((B, N, HS), np.float32)
    rows = np.arange(NHALF)
    for core in range(8):
        b, s = core // 2, core % 2
        bin_of, slot_of = metas[core]
        dump = res.results[core]["acc"]          # [128, nbins+1, 64]
        out[b, s * NHALF:(s + 1) * NHALF] = dump[slot_of[rows], bin_of[rows]]
    return out


# revision 19
# speedup vs baseline: 1.1769x; 1.0252x over previous
"""Neighbor aggregation (gnn message passing) Bass kernel for Trainium2.

out[b, i] = sum_{e: src[e]==i} w[e] * H[b, dst[e]]   (per batch b)

8 NeuronCores: core = 2*b + s handles batch b, src-half s (output rows
[s*25000, (s+1)*25000)).

Strategy ("bin-packed one-hot scatter"):
 - The only per-edge data-dependent hardware mechanism is the SWDGE
   dma_gather, whose Q7 descriptor generation costs ~7.9 ns/token on one
   queue and is the hard floor.  The previous kernel paid that floor TWICE
   (gather + dma_scatter_add).  This kernel pays it once — the
   scatter/segment-sum runs on the otherwise-idle Tensor engine as one-hot
   matmuls — and spreads desc-gen over all 4 SWDGE queues (concurrent Q7
   generation contexts, ~3.3 ns/token effective).  8192-token calls need
   single_packet=False (64 desc/engine/packet limit crashes otherwise).
 - Host packs the 25000 output rows of each core into NBINS bins of <=128
   sources, balancing per-bin token counts for BOTH dst-half phases
   (<= TPB*128 tokens per bin per phase).  The resulting tile->bin map is a
   compile-time constant shared by all 8 SPMD cores; all per-core variation
   (gather indices, weights, slot-in-bin ids) is data.
 - Device: per 8192-token chunk: SWDGE dma_gather (HBM H rows -> SBUF
   token-major f32), DVE multiply by w (f32 -> bf16), then per 128-token
   tile: DVE is_equal against a constant iota row builds the one-hot
   [token, slot] matrix; TensorE matmul-accumulates one-hot.T @ msgs into a
   per-bin PSUM tile; after the bin's last tile, DVE adds PSUM into the SBUF
   accumulator [128 slots, NBINS, 64].  Host un-permutes (slot, bin) -> row.
 - Pad tokens gather row 0 with w=0 and slot=-1 (no one-hot match), so they
   are exact no-ops.  Both phases of a source share (slot, bin), so phase
   partials merge in the accumulator with no extra pass.
"""

import os
import sys

sys.path.insert(0, "/opt/trn_rl_repo")

import numpy as np
import ml_dtypes

import concourse.bacc as bacc
import concourse.mybir as mybir
import concourse.tile as tile
from concourse.bass_utils import run_bass_kernel_spmd

B, N, E, HS = 4, 50000, 800000, 64
NHALF = N // 2                  # 25000 output rows per core
CH = 8192                       # tokens per gather chunk
TPB = 7                         # tiles (of 128 tokens) per bin per phase

LAST_RESULT = {}


def _chunk_sizes(nch_per_phase):
    """Per-phase chunk sizes: graduated ramp (4x2048, 2x4096) first resp. last
    so the 4 SWDGE queue contexts fill/drain staggered instead of all big
    generations starting at once; same total tokens."""
    ramp = [CH // 4] * 4 + [CH // 2] * 2          # = 2*CH
    sizes_a = ramp + [CH] * (nch_per_phase - 2)
    sizes_b = [CH] * (nch_per_phase - 2) + ramp[::-1]
    return sizes_a, sizes_b


def build(nc, nbins, nch_per_phase):
    f32 = mybir.dt.float32
    bf16 = mybir.dt.bfloat16
    i16 = mybir.dt.int16
    sizes_a, sizes_b = _chunk_sizes(nch_per_phase)
    sizes = sizes_a + sizes_b
    nch = len(sizes)
    tiles_per_phase = nch_per_phase * (CH // 128)
    real_tiles = nbins * TPB    # remaining tiles of the phase are dummies

    h_d = nc.dram_tensor("h", [N, HS], f32, kind="ExternalInput")
    gidx_d = nc.dram_tensor("gidx", [nch, 128, CH // 16], i16, kind="ExternalInput")
    wl_d = nc.dram_tensor("wl", [nch, 128, CH // 128], f32, kind="ExternalInput")
    scol_d = nc.dram_tensor("scol", [nch, 128, CH // 128], bf16, kind="ExternalInput")
    iotab_d = nc.dram_tensor("iotab", [128, 128], bf16, kind="ExternalInput")
    acc_d = nc.dram_tensor("acc", [128, nbins + 1, HS], f32, kind="ExternalOutput")

    with tile.TileContext(nc) as tc:
        with tc.tile_pool(name="res", bufs=1) as res, \
             tc.tile_pool(name="psum", bufs=8, space="PSUM") as pp, \
             tc.tile_pool(name="work", bufs=4) as wp, \
             tc.tile_pool(name="oh", bufs=4) as ohp:
            iotab = res.tile([128, 128], bf16, tag="iotab")
            nc.sync.dma_start(iotab[:], iotab_d[:])
            acc = res.tile([128, nbins + 1, HS], f32, tag="acc")
            nc.vector.memset(acc[:], 0.0)

            ps = None
            off = 0                     # token offset within the phase
            for c, size in enumerate(sizes):
                phase = 0 if c < len(sizes_a) else 1
                if c == len(sizes_a):
                    off = 0
                h_ap = h_d[:][phase * NHALF:(phase + 1) * NHALF, :]
                gi = wp.tile([128, size // 16], i16, tag="gi")
                wl = wp.tile([128, size // 128], f32, tag="wl")
                sc = wp.tile([128, size // 128], bf16, tag="sc")
                nc.sync.dma_start(gi[:], gidx_d[c][:, :size // 16])
                nc.sync.dma_start(wl[:], wl_d[c][:, :size // 128])
                nc.sync.dma_start(sc[:], scol_d[c][:, :size // 128])

                msgs = wp.tile([128, size // 128, HS], f32, tag="msgs")
                nc.gpsimd.dma_gather(
                    out_ap=msgs[:],
                    in_ap=h_ap,
                    idxs_ap=gi[:],
                    num_idxs=size,
                    num_idxs_reg=size,
                    elem_size=HS,
                    single_packet=False,
                    queue_num=c % 4,
                )
                msgsb = wp.tile([128, size // 128, HS], bf16, tag="msgsb")
                nc.vector.tensor_tensor(
                    out=msgsb[:],
                    in0=msgs[:],
                    in1=wl[:].unsqueeze(2).broadcast_to([128, size // 128, HS]),
                    op=mybir.AluOpType.mult,
                )

                ntile = size // 128
                tau0 = off // 128
                for j0 in range(0, ntile, 8):
                    nb = min(8, ntile - j0)
                    oh = ohp.tile([128, 8, 128], bf16, tag="oh")
                    nc.vector.tensor_tensor(
                        out=oh[:, :nb],
                        in0=sc[:, j0:j0 + nb].unsqueeze(2).broadcast_to([128, nb, 128]),
                        in1=iotab[:].unsqueeze(1).broadcast_to([128, nb, 128]),
                        op=mybir.AluOpType.is_equal,
                    )
                    for j in range(j0, j0 + nb):
                        tau = tau0 + j                          # tile idx in phase
                        if tau < real_tiles:
                            bin_, pos = tau // TPB, tau % TPB
                            last = pos == TPB - 1
                        else:
                            bin_, pos = nbins, tau - real_tiles  # dummy bin
                            last = tau == tiles_per_phase - 1
                        if pos == 0:
                            ps = pp.tile([128, HS], f32, tag="ps")
                        nc.tensor.matmul(
                            ps[:], oh[:, j - j0], msgsb[:, j],
                            start=(pos == 0), stop=last,
                        )
                        if last:
                            nc.vector.tensor_tensor(
                                out=acc[:, bin_], in0=acc[:, bin_], in1=ps[:],
                                op=mybir.AluOpType.add,
                            )
                            # bins complete in ascending order during phase B:
                            # stream finished 32-bin blocks out early
                            if phase == 1 and bin_ < nbins and bin_ % 32 == 31:
                                nc.sync.dma_start(
                                    acc_d[:][:, bin_ - 31:bin_ + 1],
                                    acc[:, bin_ - 31:bin_ + 1],
                                )
                off += size

            rem = nbins - (nbins // 32) * 32
            nc.sync.dma_start(
                acc_d[:][:, nbins - rem:], acc[:, nbins - rem:]
            )
    return nc


_COMPILED = {}


def _get_compiled(nbins, nch_per_phase):
    key = (nbins, nch_per_phase)
    if key not in _COMPILED:
        nc = bacc.Bacc(
            "TRN2", target_bir_lowering=False, debug=False, num_swdge_queues=4
        )
        build(nc, nbins, nch_per_phase)
        nc.compile()
        _COMPILED[key] = nc
    return _COMPILED[key]


def _pack_bins(dA, dB, nbins, cap):
    """Assign each source to a bin s.t. per-bin source count <=128 and
    per-bin token sums <= cap in BOTH phases.  Returns (bin, slot) per
    source, or None if infeasible."""
    nsrc = dA.shape[0]
    order = np.argsort(-(dA + dB), kind="stable")
    loadA = np.zeros(nbins, np.int64)
    loadB = np.zeros(nbins, np.int64)
    cnt = np.zeros(nbins, np.int64)
    bin_of = np.empty(nsrc, np.int64)
    slot_of = np.empty(nsrc, np.int64)
    for s in order:
        headA = cap - loadA - dA[s]
        headB = cap - loadB - dB[s]
        score = np.minimum(headA, headB)
        score[cnt >= 128] = -1
        b = int(np.argmax(score))
        if score[b] < 0:
            return None
        bin_of[s] = b
        slot_of[s] = cnt[b]
        loadA[b] += dA[s]
        loadB[b] += dB[s]
        cnt[b] += 1
    return bin_of, slot_of


def _wrap16(idx, n):
    a = idx.reshape(n // 16, 16).T.astype(np.int16)   # [16, n//16]
    return np.ascontiguousarray(np.tile(a, (8, 1)))   # [128, n//16]


def _core_edges(src, dst, w, s):
    sel = (src >= NHALF) == bool(s)
    srcs = (src[sel] - s * NHALF).astype(np.int64)
    dsts = dst[sel].astype(np.int64)
    ws = w[sel].astype(np.float32)
    phase = (dsts >= NHALF).astype(np.int64)
    dloc = dsts - phase * NHALF
    return srcs, dloc, ws, phase


def _prep_core(srcs, dloc, ws, phase, bin_of, slot_of, nbins, nch_per_phase):
    """Build gidx/wl/scol chunk arrays for one core (batch half s)."""
    cap = TPB * 128
    ntok = nch_per_phase * CH
    g_all = np.zeros((2, ntok), np.int64)
    w_all = np.zeros((2, ntok), np.float32)
    s_all = np.full((2, ntok), -1.0, np.float32)

    for ph in range(2):
        m = phase == ph
        sp, dp, wp_ = srcs[m], dloc[m], ws[m]
        # order edges by bin: position = bin base + running offset within bin
        b = bin_of[sp]
        order = np.argsort(b, kind="stable")
        sp, dp, wp_, b = sp[order], dp[order], wp_[order], b[order]
        cnts = np.bincount(b, minlength=nbins)
        starts = np.concatenate([[0], np.cumsum(cnts[:-1])])
        offs = np.arange(sp.shape[0]) - np.repeat(starts, cnts)
        pos = b * cap + offs
        assert (offs < cap).all()
        g_all[ph, pos] = dp
        w_all[ph, pos] = wp_
        s_all[ph, pos] = slot_of[sp]

    sizes_a, sizes_b = _chunk_sizes(nch_per_phase)
    nch = len(sizes_a) + len(sizes_b)
    gidx = np.zeros((nch, 128, CH // 16), np.int16)
    wl = np.zeros((nch, 128, CH // 128), np.float32)
    s_out = np.full((nch, 128, CH // 128), -1.0, np.float32)
    c = 0
    for ph, sizes in ((0, sizes_a), (1, sizes_b)):
        off = 0
        for size in sizes:
            tok = slice(off, off + size)
            gidx[c, :, :size // 16] = _wrap16(g_all[ph, tok], size)
            # token t of chunk -> [t % 128, t // 128]
            wl[c, :, :size // 128] = w_all[ph, tok].reshape(size // 128, 128).T
            s_out[c, :, :size // 128] = s_all[ph, tok].reshape(size // 128, 128).T
            off += size
            c += 1
    scol = s_out.astype(ml_dtypes.bfloat16)
    return {"gidx": gidx, "wl": wl, "scol": scol}


def kernel(**inputs):
    H = np.ascontiguousarray(np.asarray(inputs["H"], np.float32))
    w = np.asarray(inputs["edge_w"], np.float32)
    src = np.asarray(inputs["edge_src"], np.int64)
    dst = np.asarray(inputs["edge_dst"], np.int64)

    cap = TPB * 128
    edges = []
    worst = 1
    for core in range(8):
        b, s = core // 2, core % 2
        srcs, dloc, ws, phase = _core_edges(src[b], dst[b], w[b], s)
        edges.append((srcs, dloc, ws, phase))
        worst = max(worst, int((phase == 0).sum()), int((phase == 1).sum()))

    # pack all cores; grow nbins until feasible everywhere
    nbins = max(-(-NHALF // 128), -(-int(worst * 1.01) // cap))
    nbins = -(-nbins // 4) * 4
    while True:
        metas = []
        for core in range(8):
            srcs, dloc, ws, phase = edges[core]
            dA = np.bincount(srcs[phase == 0], minlength=NHALF)
            dB = np.bincount(srcs[phase == 1], minlength=NHALF)
            res = _pack_bins(dA, dB, nbins, cap)
            if res is None:
                break
            metas.append(res)
        if len(metas) == 8:
            break
        nbins += 4
    nch_per_phase = -(-(nbins * cap) // CH)

    iotab = np.tile(np.arange(128), (128, 1)).astype(ml_dtypes.bfloat16)

    in_maps = []
    for core in range(8):
        b = core // 2
        srcs, dloc, ws, phase = edges[core]
        bin_of, slot_of = metas[core]
        m = _prep_core(srcs, dloc, ws, phase, bin_of, slot_of, nbins, nch_per_phase)
        m["h"] = H[b]
        m["iotab"] = iotab
        in_maps.append(m)

    nc = _get_compiled(nbins, nch_per_phase)
    trace = bool(int(os.environ.get("GNN_TRACE", "0")))
    res = run_bass_kernel_spmd(nc, in_maps, list(range(8)), trace=trace)
    LAST_RESULT["exec_time_ns"] = res.exec_time_ns
    LAST_RESULT["res"] = res

    out = np.empty((B, N, HS), np.float32)
    rows = np.arange(NHALF)
    for core in range(8):
        b, s = core // 2, core % 2
        bin_of, slot_of = metas[core]
        dump = res.results[core]["acc"]          # [128, nbins+1, 64]
        out[b, s * NHALF:(s + 1) * NHALF] = dump[slot_of[rows], bin_of[rows]]
    return out
